# revision 24
# baseline (speedup 1.0000x reference)
"""Dot-product attention (B=32, S=2048, D=64, per-batch key masking) on 8 trn2 cores.

Strategy: split each batch into two q-half tasks (64 tasks of 1024 queries).
Task cost is proportional to ceil(valid_len/128) key chunks -- fully masked
chunks contribute exactly 0 (exp(-1e6) == 0) and are skipped. Tasks are
sorted by chunk count and packed into 8 slots x 8 cores; each slot's chunk
count is baked into the compiled kernel as the max over the 8 cores at that
slot, so the SPMD instruction stream is shared while per-core data differs.

Q^T and K^T are pre-transposed on the host (free numpy work) and augmented
with a 65th contraction row: kT row 64 holds the key mask (0 or -1e6) and
qT row 64 holds ones, so matmul1 (scores^T[k, q] = K_chunk @ Q^T, contraction
65) adds the mask bias directly into the scores. The ScalarE exp then needs
no per-chunk bias, letting one activation instruction span three 512-query
score segments ([128, 1536] across 3 PSUM banks), which amortizes its
~185ns access-latency overhead. exp output (bf16) is the *stationary*
operand of matmul2 with V chunks moving, so out[q, d] accumulates directly
in PSUM in its final orientation; per-q-block ones-matmuls accumulate the
softmax denominators into the second acc bank. Finalize: one DVE reciprocal
+ broadcast multiplies, DMA out.

The ScalarE exp is the critical engine; the rest keeps it saturated: all
loads issue in a preamble (unique SBUF tiles per slot, gpsimd DMAs casting
f32->bf16 in flight), scores PSUM is double-buffered at 3 banks each,
PSUM acc zeroing is done by zero-matmuls on the PE, and dummy ACT/PE work
at t=0 preloads the exp table and ramps the PE clock during the load phase.

Q columns and the output rows use a 4-way interleave (q = 512t + 4p + four)
so the output DMA writes >=512B descriptors.
"""

import sys

import numpy as np

_TRN_REPO = "/opt/trn_rl_repo"
if _TRN_REPO not in sys.path:
    sys.path.insert(0, _TRN_REPO)

B, S, D = 32, 2048, 64
N_CORES = 8
N_SLOTS = 8  # tasks per core (one per slot)
QLEN = 1024  # queries per task (half batch)
NCHUNK_MAX = S // 128  # 16
NEG = -1000000.0

_CACHE = {}


def _build_nc(ncaps):
    import concourse.bacc as bacc
    import concourse.bass as bass
    import concourse.mybir as mybir
    import concourse.tile as tile

    f32 = mybir.dt.float32
    bf16 = mybir.dt.bfloat16
    Exp = mybir.ActivationFunctionType.Exp

    ncap0 = ncaps[0]
    nc = bacc.Bacc()

    # qT is 384 columns wider than QLEN: for slot 0 those columns carry the
    # first three kT chunks so one DMA serves the whole startup-critical chain
    qt_d = nc.dram_tensor(
        "qT", [N_SLOTS, D + 1, QLEN + 384], f32, kind="ExternalInput"
    )
    kt_d = nc.dram_tensor(
        "kT", [N_SLOTS, D + 1, ncap0 * 128], f32, kind="ExternalInput"
    )
    v_d = nc.dram_tensor("v", [N_SLOTS, ncap0, 128, D], f32, kind="ExternalInput")
    out_d = nc.dram_tensor("out", [N_SLOTS, QLEN, D], f32, kind="ExternalOutput")

    with tile.TileContext(nc) as tc:
        with (
            tc.tile_pool(name="const", bufs=1) as constp,
            tc.tile_pool(name="ld", bufs=1) as ldp,
            tc.tile_pool(name="expp", bufs=4) as expp,
            tc.tile_pool(name="fin", bufs=2) as finp,
            tc.tile_pool(name="psc", bufs=2, space="PSUM") as psc,
            tc.tile_pool(name="pacc", bufs=1, space="PSUM") as pacc,
        ):
            ones = constp.tile([128, 1], bf16, name="ones")
            nc.vector.memset(ones[:], 1.0)
            zsrc = constp.tile([128, 512], bf16, name="zsrc")
            nc.vector.memset(zsrc[:], 0.0)

            # Preload the exp table set during the DMA phase (first real
            # activation would otherwise eat the ~1.3us table load).
            dummy = constp.tile([128, 1], bf16, name="dummy")
            nc.scalar.activation(dummy[:], ones[:], Exp, bias=0.0, scale=1.0)

            # PE p-state warmup: dependent zero-matmuls ramp the clock from
            # cold while the first loads are in flight.
            wsc = psc.tile([128, 1536], f32, name="wsc", tag="sc")
            for _ in range(6):
                nc.tensor.matmul(
                    wsc[:, 0:512], zsrc[:, 0:128], zsrc[:], start=True, stop=True,
                    skip_group_check=True,
                )

            # ---- preamble: all loads, unique SBUF tiles per slot ----
            # (f32 -> bf16 cast in flight on gpsimd-initiated DMAs; host
            # provides pre-transposed mask-augmented qT/kT)
            qts, kts, vts = [], [], []
            for j in range(N_SLOTS):
                ncap = ncaps[j]
                qt = ldp.tile([D + 1, QLEN + 384], bf16, name=f"qt{j}", tag=f"qt{j}")
                kt = ldp.tile(
                    [D + 1, ncap0 * 128], bf16, name=f"kt{j}", tag=f"kt{j}"
                )
                if j == 0:
                    # one DMA covers qT plus kT chunks 0-1 (packed into the
                    # extra qT columns on the host) -- the whole first-exp
                    # dependency chain arrives with a single Pool DGE pass
                    nc.gpsimd.dma_start(qt[:], qt_d[j])
                    nc.gpsimd.dma_start(
                        kt[:, 0 : ncap * 128], kt_d[j, :, 0 : ncap * 128]
                    )
                else:
                    nc.gpsimd.dma_start(
                        kt[:, 0 : ncap * 128], kt_d[j, :, 0 : ncap * 128]
                    )
                    nc.gpsimd.dma_start(qt[:, 0:QLEN], qt_d[j, :, 0:QLEN])
                vt = ldp.tile([128, ncap0, D], bf16, name=f"vt{j}", tag=f"vt{j}")
                nc.gpsimd.dma_start(
                    vt[:, 0:ncap, :], v_d[j, 0:ncap].rearrange("c p d -> p c d")
                )
                qts.append(qt), kts.append(kt), vts.append(vt)

            for j in range(N_SLOTS):
                ncap = ncaps[j]
                qt, kt, vt = qts[j], kts[j], vts[j]

                # acc layout: bank 0 = 8 q-blocks x 64 output cols (uniform
                # stride), bank 1 cols 512..519 = the 8 denominators.
                # Zero both via PE zero-matmuls (start=True marks the whole
                # bank pending; the write itself clears+zeroes it), then all
                # accumulating matmuls use start=False onto clean zeros --
                # per-region start=True would wipe sibling groups' chunk-0
                # contributions (pending-zero is bank-granular).
                acc = pacc.tile([128, 1024], f32, name="acc", tag="acc")
                rc = finp.tile([128, 8], f32, name="rc", tag="rc")
                outsb = finp.tile([128, 8 * D], f32, name="outsb", tag="outsb")

                def finalize_half(h):
                    # out[q, d] = acc_v[q, d] * (1/den[q]) for q-half h, then
                    # DMA that half out. The h=0 half finishes mid-slot (see
                    # segment order below), so only h=1 sits in the tail.
                    nc.vector.reciprocal(
                        rc[:, 4 * h : 4 * (h + 1)], acc[:, 512 + 4 * h : 516 + 4 * h]
                    )
                    for jj in range(4 * h, 4 * (h + 1)):
                        nc.vector.tensor_scalar_mul(
                            outsb[:, D * jj : D * (jj + 1)],
                            acc[:, 64 * jj : 64 * (jj + 1)],
                            rc[:, jj : jj + 1],
                        )
                    nc.sync.dma_start(
                        out_d[j, 512 * h : 512 * (h + 1)].rearrange(
                            "(p four) d -> p four d", p=128
                        ),
                        outsb[:, 256 * h : 256 * (h + 1)].rearrange(
                            "p (four d) -> p four d", d=D
                        ),
                    )

                # score stream: segments (c, h) of 512 queries; 3 segments
                # share one [128, 1536] PSUM tile and one exp instruction.
                # h-major order: all q-half-0 segments first, so that half's
                # output finalizes and DMAs while half 1 still streams.
                # Chunks 0-2 of slot 0 live in the qt tile (combined load).
                def ktc_ap(c):
                    if j == 0 and c < 3:
                        return qt[:, QLEN + 128 * c : QLEN + 128 * (c + 1)]
                    return kt[:, 128 * c : 128 * (c + 1)]

                segs = [(c, 0) for c in range(ncap)] + [(c, 1) for c in range(ncap)]
                h0_group = (ncap - 1) // 3  # group holding the last h=0 segment
                for gi, g0 in enumerate(range(0, len(segs), 3)):
                    g = segs[g0 : g0 + 3]
                    w = 512 * len(g)
                    sc = psc.tile([128, 1536], f32, name="sc", tag="sc")
                    for i, (c, h) in enumerate(g):
                        nc.tensor.matmul(
                            sc[:, 512 * i : 512 * (i + 1)],
                            ktc_ap(c),
                            qt[:, 512 * h : 512 * (h + 1)],
                            start=True,
                            stop=True,
                        )
                    ex = expp.tile([128, 1536], bf16, name="ex", tag="ex")
                    nc.scalar.activation(
                        ex[:, 0:w], sc[:, 0:w], Exp, bias=0.0, scale=0.125
                    )
                    if g0 == 0:
                        # zero the acc banks via PE zero-matmuls (start=True
                        # marks the whole bank pending; the write itself
                        # clears+zeroes it). Emitted after the first exp so
                        # they don't stall the first matmul1s behind the
                        # previous slot's finalize (WAR on acc).
                        nc.tensor.matmul(
                            acc[:, 0:512], zsrc[:, 0:128], zsrc[:], start=True,
                            stop=True, skip_group_check=True,
                        )
                        nc.tensor.matmul(
                            acc[:, 512:520], zsrc[:, 0:128], zsrc[:, 0:8],
                            start=True, stop=True, skip_group_check=True,
                        )
                    # accumulating matmuls use start=False onto the zeroed
                    # banks -- per-region start=True would wipe sibling
                    # groups' first writes (pending-zero is bank-granular).
                    # Denominator matmuls go first so the reciprocals start
                    # as early as possible.
                    for kind in ("den", "v"):
                        for i, (c, h) in enumerate(g):
                            for u in range(4):
                                jj = 4 * h + u
                                exj = ex[
                                    :, 512 * i + 128 * u : 512 * i + 128 * (u + 1)
                                ]
                                if kind == "v":
                                    nc.tensor.matmul(
                                        acc[:, 64 * jj : 64 * (jj + 1)],
                                        exj,
                                        vt[:, c, :],
                                        start=False,
                                        stop=False,
                                        skip_group_check=True,
                                    )
                                else:
                                    nc.tensor.matmul(
                                        acc[:, 512 + jj : 513 + jj],
                                        exj,
                                        ones[:],
                                        start=False,
                                        stop=False,
                                        skip_group_check=True,
                                    )
                    if gi == h0_group:
                        finalize_half(0)
                finalize_half(1)

    nc.compile()
    return nc


def _get_nc(ncaps):
    key = tuple(ncaps)
    if key not in _CACHE:
        _CACHE[key] = _build_nc(key)
    _CACHE["last"] = _CACHE[key]
    return _CACHE[key]


def make_schedule(valid_lens):
    """Sort half-batch tasks by chunk count; slot j is baked to the max count
    of its 8 tasks (ranks 8j..8j+7); core i takes rank 8j+i."""
    valid_lens = np.asarray(valid_lens)
    nch = np.maximum(1, -(-valid_lens // 128))  # ceil, >= 1
    tasks = sorted(
        [(int(nch[b]), b, h) for b in range(B) for h in range(2)],
        key=lambda t: (-t[0], t[1], t[2]),
    )
    ncaps = tuple(tasks[8 * j][0] for j in range(N_SLOTS))
    assign = [[tasks[8 * j + i] for j in range(N_SLOTS)] for i in range(N_CORES)]
    return ncaps, assign


def make_in_maps(queries, keys, values, valid_lens):
    queries = np.ascontiguousarray(np.asarray(queries, dtype=np.float32))
    keys = np.ascontiguousarray(np.asarray(keys, dtype=np.float32))
    values = np.ascontiguousarray(np.asarray(values, dtype=np.float32))
    valid_lens = np.asarray(valid_lens, dtype=np.int32)

    ncaps, assign = make_schedule(valid_lens)
    ncap0 = ncaps[0]
    kc = keys.reshape(B, NCHUNK_MAX, 128, D)
    vc = values.reshape(B, NCHUNK_MAX, 128, D)

    in_maps = []
    for i in range(N_CORES):
        qt_h = np.zeros((N_SLOTS, D + 1, QLEN + 384), np.float32)
        kt_h = np.zeros((N_SLOTS, D + 1, ncap0 * 128), np.float32)
        v_h = np.zeros((N_SLOTS, ncap0, 128, D), np.float32)
        for j, (cost, b, h) in enumerate(assign[i]):
            ncap = ncaps[j]
            # qT column q' = 128*jj + p maps to q = 512*(jj//4) + 4*p + jj%4
            # (4-way interleave so the output DMA writes 512B+ descriptors);
            # row 64 = ones (multiplies the kT mask row into the scores)
            qs = queries[b, h * QLEN : (h + 1) * QLEN]  # [1024, 64]
            qt_h[j, :D, 0:QLEN] = (
                qs.reshape(2, 128, 4, D).transpose(3, 0, 2, 1).reshape(D, QLEN)
            )
            qt_h[j, D, 0:QLEN] = 1.0
            # kT rows 0..63 = K^T (natural key order); row 64 = key mask
            # bias (0 if key < valid_len else -1e6)
            kt_h[j, :D, 0 : ncap * 128] = (
                kc[b, :ncap].transpose(2, 0, 1).reshape(D, ncap * 128)
            )
            kt_h[j, D, 0 : ncap * 128] = np.where(
                np.arange(ncap * 128) < valid_lens[b], 0.0, NEG
            )
            v_h[j, :ncap] = vc[b, :ncap]
            if j == 0:
                # pack kT chunks 0-2 into the extra qT columns so slot 0's
                # startup-critical data arrives in a single DMA
                w0 = min(3, ncap) * 128
                qt_h[0, :, QLEN : QLEN + w0] = kt_h[0, :, 0:w0]
        in_maps.append({"qT": qt_h, "kT": kt_h, "v": v_h})
    return ncaps, assign, in_maps


def run_on_device(ncaps, in_maps, trace=False):
    from concourse.bass_utils import run_bass_kernel_spmd

    nc = _get_nc(ncaps)
    return run_bass_kernel_spmd(
        nc, in_maps, core_ids=list(range(N_CORES)), trace=trace
    )


def assemble_out(assign, results):
    out = np.empty((B, S, D), np.float32)
    for i in range(N_CORES):
        o = results[i]["out"]
        for j, (cost, b, h) in enumerate(assign[i]):
            # the output DMA already un-permutes the q interleave
            out[b, h * QLEN : (h + 1) * QLEN] = o[j]
    return out


def kernel(**inputs):
    ncaps, assign, in_maps = make_in_maps(
        inputs["queries"], inputs["keys"], inputs["values"], inputs["valid_lens"]
    )
    res = run_on_device(ncaps, in_maps, trace=False)
    return assemble_out(assign, res.results)


if __name__ == "__main__":
    _build_nc((16, 13, 9, 7, 7, 4, 3, 2))
    print("build OK")
